# revision 1
# baseline (speedup 1.0000x reference)
"""Block-dequant linear kernel for TRN2 (8 NeuronCores).

Computes y = x @ (weight_q * block_scale).T with
  x:        [64, 7168]  f32
  weight_q: [18432, 7168] f32 (block-quantized codes)
  scale:    [144, 56]   f32 (one scale per 128x128 block)

Sharding: row-parallel over out_features. Each of the 8 cores gets a
[2304, 7168] slice of weight_q and an [18, 56] slice of scale; x is
replicated; per-core outputs y_c = [64, 2304] are concatenated on host.

Per-core kernel (all fp32 in HBM):
  1. Load x, transpose 128-col blocks on the PE (identity matmul) to
     build xT [7168, 64] laid out as 56 tiles of [128, 64] in SBUF.
  2. Broadcast scale values to all 128 partitions with a K=1 outer
     product matmul: S_b[128, 1008] = ones[128,1] @ s_flat[1, 1008].
  3. Stream W in [128, 1024] tiles (natural [o, i] layout, contiguous
     DMA), PE-transpose each 128x128 block into PSUM, then evacuate
     PSUM->SBUF on the vector engine with a fused per-block dequant
     multiply (scale broadcast via stride-0 access pattern).
  4. Accumulate y[64, o_chunk] = sum_ib xT_ib.T @ wT_ib over the 56
     contraction blocks in PSUM, evacuate, DMA out.

float32r (reduced-precision fp32 matmul mode, 4x faster moving stream)
is used for the main matmul and the W transposes when enabled.
"""

import sys

import numpy as np

import concourse.bass as bass  # noqa: E402
from concourse import bacc  # noqa: E402
import concourse.mybir as mybir  # noqa: E402
import concourse.tile as tile  # noqa: E402
from concourse.bass_utils import run_bass_kernel_spmd  # noqa: E402
from concourse.masks import make_identity  # noqa: E402

TOKENS = 64
IN_F = 7168
OUT_F = 18432
N_CORES = 8
O_PER = OUT_F // N_CORES  # 2304
OB = O_PER // 128  # 18 o-blocks per core
IBC = IN_F // 128  # 56 i-blocks
# o-chunks: PSUM accumulation tile width (max 512 f32 moving operand)
CHUNKS = [(0, 512), (512, 512), (1024, 512), (1536, 512), (2048, 256)]
IB_GROUP = 14  # i-blocks per W DMA tile
ACT_EVERY = 3  # every Nth i-block evacuates on ACT (0 = never)


def build_nc(use_f32r_mm: bool = True, use_f32r_tr: bool = True) -> bass.Bass:
    f32 = mybir.dt.float32
    f32r = mybir.dt.float32r
    mm_dt = f32r if use_f32r_mm else f32
    tr_dt = f32r if use_f32r_tr else f32
    if use_f32r_tr:
        assert use_f32r_mm, "f32r transposes require f32r matmul"

    nc = bacc.Bacc()
    x_h = nc.dram_tensor("x", [TOKENS, IN_F], f32, kind="ExternalInput")
    w_h = nc.dram_tensor("w", [O_PER, IN_F], tr_dt, kind="ExternalInput")
    # scale pre-broadcast on host to all 128 partitions: sb[p, ob*IBC+ib]
    sb_h = nc.dram_tensor("sb", [128, OB * IBC], f32, kind="ExternalInput")
    y_h = nc.dram_tensor("y", [TOKENS, O_PER], f32, kind="ExternalOutput")

    with tile.TileContext(nc) as tc:
        with tc.tile_pool(name="const", bufs=1) as cpool:
            ident = cpool.tile([128, 128], f32)
            make_identity(nc, ident)
            if tr_dt is f32:
                ident_tr = ident
            else:
                # memset/affine_select can't emit f32r; DVE copy rounds
                ident_tr = cpool.tile([128, 128], tr_dt, name="ident_tr")
                nc.vector.tensor_copy(out=ident_tr[:, :], in_=ident[:, :])

            # --- scale broadcast table S_b[p, ob*IBC+ib] = s[ob, ib] ---
            s_b = cpool.tile([128, OB * IBC], f32)
            nc.sync.dma_start(out=s_b[:, :], in_=sb_h[:, :])
            s_b3 = s_b[:, :].rearrange("p (ob ib) -> p ob ib", ib=IBC)

            # --- x load + transpose to xT tiles [128, 64] ---
            # separate tile per DMA so each transpose waits on exactly one
            # DMA queue (LDW instructions have a tiny sync-wait budget)
            xw = IN_F // 8  # 896 = 7 blocks
            x_parts = []
            for xc in range(8):
                xp = cpool.tile([TOKENS, xw], f32, name=f"xp{xc}")
                nc.sync.dma_start(
                    out=xp[:, :], in_=x_h[:, xc * xw : (xc + 1) * xw]
                )
                x_parts.append(xp)
            x_t = cpool.tile([128, IBC * TOKENS], mm_dt)
            with tc.tile_pool(name="xpsum", bufs=4, space="PSUM") as xpp:
                for ib in range(IBC):
                    pt = xpp.tile([128, TOKENS], f32)
                    nc.tensor.transpose(
                        pt,
                        x_parts[ib // 7][:, (ib % 7) * 128 : (ib % 7 + 1) * 128],
                        ident[:TOKENS, :TOKENS],
                    )
                    nc.vector.tensor_copy(
                        out=x_t[:, ib * TOKENS : (ib + 1) * TOKENS], in_=pt
                    )

            # --- main loop ---
            with (
                tc.tile_pool(name="wpool", bufs=8) as wpool,
                tc.tile_pool(name="wtpool", bufs=3) as wtpool,
                tc.tile_pool(name="opool", bufs=2) as opool,
                tc.tile_pool(name="accp", bufs=2, space="PSUM") as accp,
                tc.tile_pool(name="trp", bufs=2, space="PSUM") as trp,
            ):
                ndma = 0
                for cbase, ch in CHUNKS:
                    nob = ch // 128
                    ob0 = cbase // 128
                    acc = accp.tile([TOKENS, 512], f32, tag="acc", name="acc")[:, :ch]
                    for ibg in range(IBC // IB_GROUP):
                        wns = []
                        for j in range(nob):
                            wn = wpool.tile([128, IB_GROUP * 128], tr_dt, tag="wn", name="wn")
                            ndma += 1
                            nc.sync.dma_start(
                                out=wn[:, :],
                                in_=w_h[
                                    (ob0 + j) * 128 : (ob0 + j + 1) * 128,
                                    ibg * IB_GROUP * 128 : (ibg + 1) * IB_GROUP * 128,
                                ],
                            )
                            wns.append(wn)
                        for ibi in range(IB_GROUP):
                            ib = ibg * IB_GROUP + ibi
                            ptile = trp.tile([128, 512], tr_dt, tag="pt", name="pt")[:, :ch]
                            for j in range(nob):
                                nc.tensor.transpose(
                                    ptile[:, j * 128 : (j + 1) * 128],
                                    wns[j][:, ibi * 128 : (ibi + 1) * 128],
                                    ident_tr[:, :],
                                )
                            wt = wtpool.tile([128, 512], mm_dt, tag="wt", name="wt")[:, :ch]
                            if ACT_EVERY and ib % ACT_EVERY == ACT_EVERY - 1:
                                # offload to the otherwise-idle ACT engine
                                for j in range(nob):
                                    nc.scalar.activation(
                                        wt[:, j * 128 : (j + 1) * 128],
                                        ptile.bitcast(f32)[
                                            :, j * 128 : (j + 1) * 128
                                        ],
                                        mybir.ActivationFunctionType.Copy,
                                        scale=s_b3[:, ob0 + j, ib : ib + 1],
                                    )
                            else:
                                sca = (
                                    s_b3[:, ob0 : ob0 + nob, ib]
                                    .unsqueeze(2)
                                    .broadcast_to([128, nob, 128])
                                )
                                nc.vector.tensor_mul(
                                    out=wt.rearrange("p (b c) -> p b c", c=128),
                                    in0=ptile.bitcast(f32).rearrange(
                                        "p (b c) -> p b c", c=128
                                    ),
                                    in1=sca,
                                )
                            nc.tensor.matmul(
                                acc,
                                lhsT=x_t[:, ib * TOKENS : (ib + 1) * TOKENS],
                                rhs=wt,
                                start=(ib == 0),
                                stop=(ib == IBC - 1),
                            )
                    ysb = opool.tile([TOKENS, 512], f32, tag="ysb", name="ysb")[:, :ch]
                    nc.any.tensor_copy(out=ysb, in_=acc)
                    nc.sync.dma_start(out=y_h[:, cbase : cbase + ch], in_=ysb)
    nc.compile()
    return nc


_NC_CACHE: dict = {}


def _get_nc(use_f32r_mm=True, use_f32r_tr=True):
    key = (use_f32r_mm, use_f32r_tr)
    if key not in _NC_CACHE:
        _NC_CACHE[key] = build_nc(*key)
    return _NC_CACHE[key]


def kernel(x, weight_q, scale, _trace=False, _f32r=(True, True)):
    x = np.ascontiguousarray(np.asarray(x, dtype=np.float32))
    weight_q = np.ascontiguousarray(np.asarray(weight_q, dtype=np.float32))
    scale = np.ascontiguousarray(np.asarray(scale, dtype=np.float32))
    nc = _get_nc(*_f32r)
    in_maps = [
        {
            "x": x,
            "w": np.ascontiguousarray(weight_q[c * O_PER : (c + 1) * O_PER]),
            "sb": np.ascontiguousarray(
                np.broadcast_to(
                    scale[c * OB : (c + 1) * OB].reshape(1, OB * IBC), (128, OB * IBC)
                )
            ),
        }
        for c in range(N_CORES)
    ]
    res = run_bass_kernel_spmd(nc, in_maps, list(range(N_CORES)), trace=_trace)
    y = np.concatenate([res.results[c]["y"] for c in range(N_CORES)], axis=1)
    if _trace:
        return y, res
    return y


if __name__ == "__main__":
    rng = np.random.default_rng(0)
    x = rng.standard_normal((TOKENS, IN_F), dtype=np.float32)
    w = rng.standard_normal((OUT_F, IN_F), dtype=np.float32)
    s = rng.random((OUT_F // 128, IN_F // 128), dtype=np.float32)
    y = kernel(x, w, s)
    print("ok", y.shape, y.dtype)



# revision 2
# speedup vs baseline: 1.8031x; 1.8031x over previous
"""Block-dequant linear kernel for TRN2 (8 NeuronCores).

Computes y = x @ (weight_q * block_scale).T with
  x:        [64, 7168]  f32
  weight_q: [18432, 7168] f32 (block-quantized codes)
  scale:    [144, 56]   f32 (one scale per 128x128 block)

Sharding: row-parallel over out_features. Each of the 8 cores gets a
[2304, 7168] slice of weight_q and an [18, 56] slice of scale; x is
replicated; per-core outputs y_c = [64, 2304] are concatenated on host.

Host-side layout prep (no device work):
  - W slice is pre-transposed to wT [7168, 2304] so the device DMAs
    contraction-major strips directly; no on-chip PE transposes.
  - x is pre-interleaved to xt[p, ib*64+t] = x[t, ib*128+p] so the
    whole stationary operand arrives in one contiguous DMA.
  - scale rows are pre-broadcast to all 128 partitions.

Per-core device kernel (all fp32 in HBM):
  for ib in 56 contraction blocks:
    1. DMA strip wT[ib*128:(ib+1)*128, :] -> [128, 2304] SBUF
    2. one DVE multiply dequantizes the strip (scale broadcast via
       stride-0 access pattern, 18 scalars per strip)
    3. 5 matmuls acc_c[64, ch] += xt_ib^T @ wdq[:, chunk] accumulate
       in PSUM (start at ib=0, stop at ib=55)
  evacuate 5 PSUM accs -> SBUF -> DMA out.

float32r (reduced-precision fp32 matmul mode, 4x faster moving stream)
is used for the matmul operands when enabled.
"""

import numpy as np

import concourse.bass as bass  # noqa: E402
from concourse import bacc  # noqa: E402
import concourse.mybir as mybir  # noqa: E402
import concourse.tile as tile  # noqa: E402
from concourse.bass_utils import run_bass_kernel_spmd  # noqa: E402

TOKENS = 64
IN_F = 7168
OUT_F = 18432
N_CORES = 8
O_PER = OUT_F // N_CORES  # 2304
OB = O_PER // 128  # 18 o-blocks per core
IBC = IN_F // 128  # 56 i-blocks
# o-chunks: PSUM accumulation tile width (max 512 f32 moving operand)
CHUNKS = [(0, 512), (512, 512), (1024, 512), (1536, 512), (2048, 256)]


def build_nc(use_f32r_mm: bool = True) -> bass.Bass:
    f32 = mybir.dt.float32
    mm_dt = mybir.dt.float32r if use_f32r_mm else f32

    nc = bacc.Bacc()
    # x pre-interleaved on host: xt[p, ib*64+t] = x[t, ib*128+p]
    xt_h = nc.dram_tensor("xt", [128, IBC * TOKENS], mm_dt, kind="ExternalInput")
    # W slice pre-transposed on host: wt[i, o]
    wt_h = nc.dram_tensor("wt", [IN_F, O_PER], mm_dt, kind="ExternalInput")
    # scale pre-broadcast on host to all 128 partitions: sb[p, ob*IBC+ib]
    sb_h = nc.dram_tensor("sb", [128, OB * IBC], f32, kind="ExternalInput")
    y_h = nc.dram_tensor("y", [TOKENS, O_PER], f32, kind="ExternalOutput")

    with tile.TileContext(nc) as tc:
        with (
            tc.tile_pool(name="const", bufs=1) as cpool,
            tc.tile_pool(name="wraw", bufs=3) as wraw,
            tc.tile_pool(name="wdq", bufs=2) as wdq,
            tc.tile_pool(name="opool", bufs=2) as opool,
            tc.tile_pool(name="accp", bufs=1, space="PSUM") as accp,
        ):
            xt = cpool.tile([128, IBC * TOKENS], mm_dt, name="xt")
            nc.sync.dma_start(out=xt[:, :], in_=xt_h[:, :])
            s_b = cpool.tile([128, OB * IBC], f32, name="sb")
            nc.sync.dma_start(out=s_b[:, :], in_=sb_h[:, :])
            s_b3 = s_b[:, :].rearrange("p (ob ib) -> p ob ib", ib=IBC)

            accs = [
                accp.tile([TOKENS, 512], f32, name=f"acc{k}")[:, :ch]
                for k, (_, ch) in enumerate(CHUNKS)
            ]

            for ib in range(IBC):
                wr = wraw.tile([128, O_PER], mm_dt, tag="wr", name="wr")
                nc.sync.dma_start(
                    out=wr[:, :], in_=wt_h[ib * 128 : (ib + 1) * 128, :]
                )
                wd = wdq.tile([128, O_PER], mm_dt, tag="wd", name="wd")
                sca = (
                    s_b3[:, :, ib].unsqueeze(2).broadcast_to([128, OB, 128])
                )
                nc.vector.tensor_mul(
                    out=wd[:, :].rearrange("p (ob c) -> p ob c", c=128),
                    in0=wr[:, :].bitcast(f32).rearrange("p (ob c) -> p ob c", c=128),
                    in1=sca,
                )
                for k, (cb, ch) in enumerate(CHUNKS):
                    nc.tensor.matmul(
                        accs[k],
                        lhsT=xt[:, ib * TOKENS : (ib + 1) * TOKENS],
                        rhs=wd[:, cb : cb + ch],
                        start=(ib == 0),
                        stop=(ib == IBC - 1),
                    )
            for k, (cb, ch) in enumerate(CHUNKS):
                ysb = opool.tile([TOKENS, 512], f32, tag="ysb", name="ysb")[:, :ch]
                nc.any.tensor_copy(out=ysb, in_=accs[k])
                nc.sync.dma_start(out=y_h[:, cb : cb + ch], in_=ysb)
    nc.compile()
    return nc


_NC_CACHE: dict = {}


def _get_nc(use_f32r_mm=True):
    key = bool(use_f32r_mm)
    if key not in _NC_CACHE:
        _NC_CACHE[key] = build_nc(key)
    return _NC_CACHE[key]


def kernel(x, weight_q, scale, _trace=False, _f32r=(True, True)):
    x = np.ascontiguousarray(np.asarray(x, dtype=np.float32))
    weight_q = np.asarray(weight_q, dtype=np.float32)
    scale = np.asarray(scale, dtype=np.float32)
    nc = _get_nc(_f32r[0])
    # xt[p, ib*64+t] = x[t, ib*128+p]
    xt = np.ascontiguousarray(
        x.reshape(TOKENS, IBC, 128).transpose(2, 1, 0).reshape(128, IBC * TOKENS)
    )
    in_maps = []
    for c in range(N_CORES):
        wt = np.ascontiguousarray(weight_q[c * O_PER : (c + 1) * O_PER].T)
        sb = np.ascontiguousarray(
            np.broadcast_to(
                scale[c * OB : (c + 1) * OB].reshape(1, OB * IBC), (128, OB * IBC)
            )
        )
        in_maps.append({"xt": xt, "wt": wt, "sb": sb})
    res = run_bass_kernel_spmd(nc, in_maps, list(range(N_CORES)), trace=_trace)
    y = np.concatenate([res.results[c]["y"] for c in range(N_CORES)], axis=1)
    if _trace:
        return y, res
    return y


if __name__ == "__main__":
    rng = np.random.default_rng(0)
    x = rng.standard_normal((TOKENS, IN_F), dtype=np.float32)
    w = rng.standard_normal((OUT_F, IN_F), dtype=np.float32)
    s = rng.random((OUT_F // 128, IN_F // 128), dtype=np.float32)
    y = kernel(x, w, s)
    print("ok", y.shape, y.dtype)


# revision 3
# speedup vs baseline: 2.4996x; 1.3863x over previous
"""Block-dequant linear kernel for TRN2 (8 NeuronCores).

Computes y = x @ (weight_q * block_scale).T with
  x:        [64, 7168]  f32
  weight_q: [18432, 7168] f32 (block-quantized codes)
  scale:    [144, 56]   f32 (one scale per 128x128 block)

Sharding: row-parallel over out_features. Each of the 8 cores gets a
[2304, 7168] slice of weight_q and an [18, 56] slice of scale; x is
replicated; per-core outputs y_c = [64, 2304] are concatenated on host.

Host-side layout prep (no device FLOPs on W beyond dtype rounding):
  - W slice is pre-transposed to wT [7168, 2304] so the device DMAs
    contraction-major strips directly; no on-chip PE transposes.
  - x is pre-interleaved to xt[p, ib*64+t] = x[t, ib*128+p] so the
    whole stationary operand arrives in one contiguous DMA.
  - scale rows are pre-broadcast to all 128 partitions.
  - in "f16"/"bf16" modes W and x are cast to 16-bit on host, halving
    HBM traffic and doubling PE/DVE throughput (tolerance is 2e-2;
    fp16 lands ~1e-3).

Per-core device kernel:
  for ib in 56 contraction blocks:
    1. DMA strip wT[ib*128:(ib+1)*128, :] -> [128, 2304] SBUF
    2. one DVE multiply dequantizes the strip (scale broadcast via
       stride-0 access pattern, 18 scalars per strip)
    3. 5 matmuls acc_c[64, ch] += xt_ib^T @ wdq[:, chunk] accumulate
       in PSUM (start at ib=0, stop at ib=55)
  evacuate 5 PSUM accs -> SBUF -> DMA out.
"""

import numpy as np

import concourse.bass as bass  # noqa: E402
from concourse import bacc  # noqa: E402
import concourse.mybir as mybir  # noqa: E402
import concourse.tile as tile  # noqa: E402
from concourse.bass_utils import run_bass_kernel_spmd  # noqa: E402

TOKENS = 64
IN_F = 7168
OUT_F = 18432
N_CORES = 8
O_PER = OUT_F // N_CORES  # 2304
OB = O_PER // 128  # 18 o-blocks per core
IBC = IN_F // 128  # 56 i-blocks
# o-chunks: PSUM accumulation tile width (max 512 f32 moving operand)
CHUNKS = [(0, 512), (512, 512), (1024, 512), (1536, 512), (2048, 256)]

_DT = {
    "f32r": (mybir.dt.float32r, np.float32),
    "f32": (mybir.dt.float32, np.float32),
    "f16": (mybir.dt.float16, np.float16),
}
try:
    import ml_dtypes

    _DT["bf16"] = (mybir.dt.bfloat16, ml_dtypes.bfloat16)
except ImportError:
    pass

WRAW_BUFS = {4: 4, 2: 6}  # by element size


def build_nc(dt_mode: str = "f16") -> bass.Bass:
    f32 = mybir.dt.float32
    mm_dt, _ = _DT[dt_mode]
    esize = 4 if dt_mode in ("f32r", "f32") else 2

    nc = bacc.Bacc()
    # x pre-interleaved on host: xt[p, ib*64+t] = x[t, ib*128+p]
    xt_h = nc.dram_tensor("xt", [128, IBC * TOKENS], mm_dt, kind="ExternalInput")
    # W slice pre-transposed on host: wt[i, o]
    wt_h = nc.dram_tensor("wt", [IN_F, O_PER], mm_dt, kind="ExternalInput")
    # scale pre-broadcast on host to all 128 partitions: sb[p, ob*IBC+ib]
    sb_h = nc.dram_tensor("sb", [128, OB * IBC], f32, kind="ExternalInput")
    y_h = nc.dram_tensor("y", [TOKENS, O_PER], f32, kind="ExternalOutput")

    with tile.TileContext(nc) as tc:
        with (
            tc.tile_pool(name="const", bufs=1) as cpool,
            tc.tile_pool(name="wraw", bufs=WRAW_BUFS[esize]) as wraw,
            tc.tile_pool(name="wdq", bufs=3) as wdq,
            tc.tile_pool(name="opool", bufs=2) as opool,
            tc.tile_pool(name="accp", bufs=1, space="PSUM") as accp,
        ):
            xt = cpool.tile([128, IBC * TOKENS], mm_dt, name="xt")
            nc.sync.dma_start(out=xt[:, :], in_=xt_h[:, :])
            s_b = cpool.tile([128, OB * IBC], f32, name="sb")
            nc.sync.dma_start(out=s_b[:, :], in_=sb_h[:, :])
            s_b3 = s_b[:, :].rearrange("p (ob ib) -> p ob ib", ib=IBC)

            accs = [
                accp.tile([TOKENS, 512], f32, name=f"acc{k}")[:, :ch]
                for k, (_, ch) in enumerate(CHUNKS)
            ]

            for ib in range(IBC):
                wr = wraw.tile([128, O_PER], mm_dt, tag="wr", name="wr")
                nc.sync.dma_start(
                    out=wr[:, :], in_=wt_h[ib * 128 : (ib + 1) * 128, :]
                )
                wd = wdq.tile([128, O_PER], mm_dt, tag="wd", name="wd")
                sca = s_b3[:, :, ib].unsqueeze(2).broadcast_to([128, OB, 128])
                in0 = wr[:, :]
                if dt_mode == "f32r":
                    in0 = in0.bitcast(f32)
                nc.vector.tensor_mul(
                    out=wd[:, :].rearrange("p (ob c) -> p ob c", c=128),
                    in0=in0.rearrange("p (ob c) -> p ob c", c=128),
                    in1=sca,
                )
                for k, (cb, ch) in enumerate(CHUNKS):
                    nc.tensor.matmul(
                        accs[k],
                        lhsT=xt[:, ib * TOKENS : (ib + 1) * TOKENS],
                        rhs=wd[:, cb : cb + ch],
                        start=(ib == 0),
                        stop=(ib == IBC - 1),
                    )
            for k, (cb, ch) in enumerate(CHUNKS):
                ysb = opool.tile([TOKENS, 512], f32, tag="ysb", name="ysb")[:, :ch]
                nc.any.tensor_copy(out=ysb, in_=accs[k])
                nc.sync.dma_start(out=y_h[:, cb : cb + ch], in_=ysb)
    nc.compile()
    return nc


_NC_CACHE: dict = {}


def _get_nc(dt_mode="f16"):
    if dt_mode not in _NC_CACHE:
        _NC_CACHE[dt_mode] = build_nc(dt_mode)
    return _NC_CACHE[dt_mode]


def kernel(x, weight_q, scale, _trace=False, _dt="f16"):
    x = np.ascontiguousarray(np.asarray(x, dtype=np.float32))
    weight_q = np.asarray(weight_q, dtype=np.float32)
    scale = np.asarray(scale, dtype=np.float32)
    nc = _get_nc(_dt)
    _, np_dt = _DT[_dt]
    # xt[p, ib*64+t] = x[t, ib*128+p]
    xt = np.ascontiguousarray(
        x.reshape(TOKENS, IBC, 128).transpose(2, 1, 0).reshape(128, IBC * TOKENS)
    ).astype(np_dt)
    in_maps = []
    for c in range(N_CORES):
        wt = np.ascontiguousarray(
            weight_q[c * O_PER : (c + 1) * O_PER].astype(np_dt).T
        )
        sb = np.ascontiguousarray(
            np.broadcast_to(
                scale[c * OB : (c + 1) * OB].reshape(1, OB * IBC), (128, OB * IBC)
            )
        )
        in_maps.append({"xt": xt, "wt": wt, "sb": sb})
    res = run_bass_kernel_spmd(nc, in_maps, list(range(N_CORES)), trace=_trace)
    y = np.concatenate([res.results[c]["y"] for c in range(N_CORES)], axis=1)
    if _trace:
        return y, res
    return y


if __name__ == "__main__":
    rng = np.random.default_rng(0)
    x = rng.standard_normal((TOKENS, IN_F), dtype=np.float32)
    w = rng.standard_normal((OUT_F, IN_F), dtype=np.float32)
    s = rng.random((OUT_F // 128, IN_F // 128), dtype=np.float32)
    y = kernel(x, w, s)
    print("ok", y.shape, y.dtype)
